# revision 12
# baseline (speedup 1.0000x reference)
"""Trainium2 Bass kernel for nn_KernelMachine (random Fourier features).

out[n,m] = sum_f sqrt(2/F) * cos(x_n . a_f + b_f) * W[f*M+m]

Strategy (data-parallel over 8 NeuronCores, N sharded, a/b/W replicated):

Host prep: x is split hi/lo in bf16 and packed (transposed) into xpack rows
on the host, so the device runs only the main pipeline. The 50 contraction
rows (ah.xh + ah.xl + al.xh + bias rows) are duplicated at partition offset
64 so the two f-chunks of each iteration run as CONCURRENT row-tiled
matmuls on disjoint PE row-groups.

Per core (N_loc=4096, D=16, F=4096, M=16), per iteration (2 f-chunks x 512 n):
  1. m1 (PE, row-tiled pair): t = (x @ a.T + b + pi/2) / (2*pi) in PSUM fp32.
     tile_position (0,0) and (64,0) -> both 512-col matmuls overlap (~250ns).
  2. DVE magic-round: k = (t + 1.5*2^23) - 1.5*2^23 (exact rint), bf16.
     (PSUM fp32 source = 1x mode; this is the pipeline bottleneck engine.)
  3. corr (PE): t -= I @ k accumulated into the same PSUM bank, giving
     s = t - rint(t) in [-0.5, 0.5] (exact Sterbenz subtraction).
  4. ACT: phi = Sin(2*pi*s) == cos(x.a + b), f32r SBUF.
  5. m2 (PE, f32r): outT[m, n] += (W*sqrt(2/F))[f,m].T @ phi[f, n]
     accumulated over the 32 f-chunks.
  6. epilogue per 512-row group: ACT copies outT [16,512] PSUM->SBUF fp32,
     straight DMA into out[16, N_loc]; the host transposes to [N_loc, 16].
     (Keeps the DVE, which is the critical engine, out of the epilogue.)
"""

import math

import numpy as np
import ml_dtypes

import concourse.bass as bass
import concourse.tile as tile
from concourse import bacc, mybir
from concourse.bass_utils import run_bass_kernel_spmd

F32 = mybir.dt.float32
F32R = mybir.dt.float32r
BF16 = mybir.dt.bfloat16
FP16 = mybir.dt.float16

N, D, F, M = 32768, 16, 4096, 16
NCORES = 8
NLOC = N // NCORES            # 4096 rows per core
FC = F // 128                 # 32 f-chunks of 128
NJ = NLOC // 512              # 8 n-groups of 512
NIT = NJ * (FC // 2)          # 128 iterations, 2 f-chunks each

MAGIC = float(np.float32(1.5 * 2 ** 23))
TWO_PI = float(2.0 * np.pi)
ROW_TILE_M1 = True

_CACHE = {}


def build_nc():
    nc = bacc.Bacc(None, target_bir_lowering=False)

    # group-major staging so each slab is one contiguous DMA
    xpack_in = nc.dram_tensor("xpack_in", [NJ, 128, 512], BF16, kind="ExternalInput")
    apack_in = nc.dram_tensor("apack_in", [4, 128, 512], BF16, kind="ExternalInput")
    wsc_in = nc.dram_tensor("wsc_in", [128, FC, M], F32R, kind="ExternalInput")
    negi_in = nc.dram_tensor("negi_in", [128, 128], BF16, kind="ExternalInput")
    out_t = nc.dram_tensor("out", [M, NLOC], F32, kind="ExternalOutput")

    with tile.TileContext(nc) as tc:
        with (
            tc.tile_pool(name="const", bufs=1) as const,
            tc.tile_pool(name="kp", bufs=6) as kp,
            tc.tile_pool(name="php", bufs=7) as php,
            tc.tile_pool(name="osb", bufs=2) as osb,
            tc.tile_pool(name="pst", bufs=3, space="PSUM") as pst,
            tc.tile_pool(name="pso", bufs=2, space="PSUM") as pso,
        ):
            # constants; DMA order puts the first iteration's operands first
            apack = const.tile([128, (FC // 2) * 128], BF16, tag="apack")
            xpack = const.tile([128, NLOC], BF16, tag="xpack")
            negi = const.tile([128, 128], BF16, tag="negi")
            wsc = const.tile([128, FC, M], F32R, tag="wsc")
            nc.sync.dma_start(out=apack[:, 0:512], in_=apack_in[0])
            nc.sync.dma_start(out=xpack[:, 0:512], in_=xpack_in[0])
            nc.sync.dma_start(out=negi, in_=negi_in[:])
            for sl in range(1, 4):
                nc.sync.dma_start(
                    out=apack[:, 512 * sl:512 * (sl + 1)], in_=apack_in[sl]
                )
            nc.sync.dma_start(out=wsc, in_=wsc_in[:])
            for g in range(1, NJ):
                nc.sync.dma_start(
                    out=xpack[:, 512 * g:512 * (g + 1)], in_=xpack_in[g]
                )

            t_tiles = {}
            k_tiles = {}
            phi_tiles = {}
            out_ps_by_j = {}

            def emit_epilogue(j):
                out_ps = out_ps_by_j.pop(j)
                outT = osb.tile([16, 512], F32, tag="outT")
                nc.scalar.copy(out=outT, in_=out_ps)
                nc.sync.dma_start(
                    out=out_t[:, 512 * j:512 * (j + 1)], in_=outT
                )

            # Software pipeline; per loop step the PE queue receives
            #   m1(it) | m2(it-3) | corr(it-1)
            # so every PE consumer is >=1 step behind its cross-engine producer.
            for it in range(NIT + 3):
                # ---- m1(it) + round(it) ----
                if it < NIT:
                    j, cp = divmod(it, FC // 2)
                    tp = pst.tile([128, 1024], F32, tag="t")
                    blk = slice(128 * cp, 128 * (cp + 1))
                    cols = slice(512 * j, 512 * (j + 1))
                    if ROW_TILE_M1:
                        nc.tensor.matmul(
                            tp[:, 0:512], apack[0:50, blk], xpack[0:50, cols],
                            start=True, stop=False, tile_position=(0, 0),
                        )
                        nc.tensor.matmul(
                            tp[:, 512:1024], apack[64:114, blk], xpack[64:114, cols],
                            start=True, stop=False, tile_position=(64, 0),
                        )
                    else:
                        nc.tensor.matmul(
                            tp[:, 0:512], apack[0:50, blk], xpack[0:50, cols],
                            start=True, stop=False,
                        )
                        nc.tensor.matmul(
                            tp[:, 512:1024], apack[64:114, blk], xpack[64:114, cols],
                            start=True, stop=False,
                        )
                    t_tiles[it] = tp
                    k_bf = kp.tile([128, 1024], BF16, tag="k")
                    nc.vector.tensor_scalar(
                        out=k_bf, in0=tp,
                        scalar1=MAGIC, scalar2=MAGIC,
                        op0=mybir.AluOpType.add, op1=mybir.AluOpType.subtract,
                    )
                    k_tiles[it] = k_bf
                # ---- m2(it-3) ----
                if 0 <= it - 3 < NIT:
                    it2 = it - 3
                    j2, cp2 = divmod(it2, FC // 2)
                    if cp2 == 0:
                        out_ps = pso.tile([16, 512], F32, tag="o")
                        out_ps_by_j[j2] = out_ps
                    out_ps = out_ps_by_j[j2]
                    phi = phi_tiles.pop(it2)
                    for h in range(2):
                        c = 2 * cp2 + h
                        nc.tensor.matmul(
                            out_ps,
                            wsc[:, c, :],
                            phi[:, 512 * h:512 * (h + 1)],
                            start=(c == 0), stop=(c == FC - 1),
                        )
                    if cp2 == FC // 2 - 1:
                        emit_epilogue(j2)
                # ---- corr(it-1) + sin(it-1) ----
                if 0 <= it - 1 < NIT:
                    it1 = it - 1
                    tp = t_tiles.pop(it1)
                    k_bf = k_tiles.pop(it1)
                    for h in range(2):
                        nc.tensor.matmul(
                            tp[:, 512 * h:512 * (h + 1)],
                            negi,
                            k_bf[:, 512 * h:512 * (h + 1)],
                            start=False, stop=True,
                        )
                    phi = php.tile([128, 1024], F32R, tag="phi")
                    nc.scalar.activation(
                        out=phi, in_=tp,
                        func=mybir.ActivationFunctionType.Sin,
                        bias=0.0, scale=TWO_PI,
                    )
                    phi_tiles[it1] = phi

    nc.finalize()
    return nc


def _host_prep(a, b, W):
    """Replicated operand packs (float64 intermediates for exact splitting)."""
    inv2pi = 1.0 / (2.0 * np.pi)
    a64 = np.asarray(a, dtype=np.float64).T * inv2pi          # [16, F]
    ah = a64.astype(ml_dtypes.bfloat16)
    al = (a64 - ah.astype(np.float64)).astype(ml_dtypes.bfloat16)
    b64 = (np.asarray(b, dtype=np.float64) + np.pi / 2.0) * inv2pi  # [F]
    bh = b64.astype(ml_dtypes.bfloat16)
    bl = (b64 - bh.astype(np.float64)).astype(ml_dtypes.bfloat16)

    # apack: 16 column-blocks of 128; block cp holds f-chunk 2cp in rows
    # 0:50 and f-chunk 2cp+1 in rows 64:114 (for the row-tiled m1 pair).
    apack = np.zeros((128, (FC // 2) * 128), dtype=ml_dtypes.bfloat16)
    for half, roff in ((0, 0), (1, 64)):
        ahh = ah.reshape(D, FC, 128)[:, half::2, :].reshape(D, -1)
        all_ = al.reshape(D, FC, 128)[:, half::2, :].reshape(D, -1)
        bhh = bh.reshape(FC, 128)[half::2, :].reshape(-1)
        bll = bl.reshape(FC, 128)[half::2, :].reshape(-1)
        apack[roff + 0:roff + 16] = ahh
        apack[roff + 16:roff + 32] = ahh
        apack[roff + 32:roff + 48] = all_
        apack[roff + 48] = bhh
        apack[roff + 49] = bll

    scale = math.sqrt(2.0 / F)
    W2 = (np.asarray(W, dtype=np.float64).reshape(F, M) * scale).astype(np.float32)
    wsc = np.ascontiguousarray(W2.reshape(FC, 128, M).transpose(1, 0, 2))

    negi = (-np.eye(128)).astype(ml_dtypes.bfloat16)
    return apack, wsc, negi


def _prep_x(x):
    """Full-N xpack [128, N] bf16: hi/lo split rows, duplicated at +64."""
    x64 = np.asarray(x, dtype=np.float64).T                   # [16, N]
    xh = x64.astype(ml_dtypes.bfloat16)
    xl = (x64 - xh.astype(np.float64)).astype(ml_dtypes.bfloat16)
    xpack = np.zeros((128, x64.shape[1]), dtype=ml_dtypes.bfloat16)
    for roff in (0, 64):
        xpack[roff + 0:roff + 16] = xh
        xpack[roff + 16:roff + 32] = xl
        xpack[roff + 32:roff + 48] = xh
        xpack[roff + 48:roff + 50] = 1.0
    return xpack


def make_in_maps(x, a, b, W):
    apack, wsc, negi = _host_prep(a, b, W)
    xpack = _prep_x(x)
    apack3 = np.ascontiguousarray(
        apack.reshape(128, 4, 512).transpose(1, 0, 2)
    )
    in_maps = []
    for i in range(NCORES):
        xp = xpack[:, i * NLOC:(i + 1) * NLOC]
        xp3 = np.ascontiguousarray(xp.reshape(128, NJ, 512).transpose(1, 0, 2))
        in_maps.append({
            "xpack_in": xp3,
            "apack_in": apack3,
            "wsc_in": wsc,
            "negi_in": negi,
        })
    return in_maps


def kernel(x, a, b, W):
    if "nc" not in _CACHE:
        _CACHE["nc"] = build_nc()
    nc = _CACHE["nc"]
    in_maps = make_in_maps(x, a, b, W)
    res = run_bass_kernel_spmd(nc, in_maps, core_ids=list(range(NCORES)))
    return np.concatenate(
        [np.ascontiguousarray(np.asarray(r["out"]).T) for r in res.results], axis=0
    )
